# revision 82
# baseline (speedup 1.0000x reference)
"""Causal self-attention (B=4, T=2048, C=1024, 16 heads) on 8 TRN2 NeuronCores.

Sharding: tensor-parallel over heads. Each core owns 2 heads (128 of the
1024 q/k/v dims): wq/wk/wv are split by rows (output dim), wo by columns.
Each core computes a full [B*T, C] partial of the output projection; the
host sums the 8 partials.

All-bf16 dataflow: x, weights, q/k/v and probabilities are bf16 (the
moving operand's dtype sets matmul speed: bf16 = 1 cycle/row at any
width). Scores s^T = K Q^T land in PSUM f32; exp (scale 1/8) runs on the
scalar engine; causal masking on the diagonal 128-blocks uses gpsimd
affine_select with fill exp(-10) (the module masks logits with -10, not
-inf). The P@V matmul is *flipped*: stationary = pexp [128k x 128q]
slice, moving = v-block [128k x 65] (64 dims + a ones column yielding the
softmax denominator) -> 65-cycle matmuls accumulating y [128q x 65] per
128-query tile. Fully-masked key blocks contribute exp(-10)*128*#blocks
to the denominator via one 1-wide matmul per group (stationary = the
constant suffix-mask matrix, moving = a constant 128.0 column); their
V-weighted numerator term (~2e-3 absolute) is dropped, which removes
the whole per-batch V-column-sum pass and its batch-boundary
dependency. V is projected directly into [token, dim] layout (stationary =
x chunk, moving = wv) so no separate transpose pass is needed. Each y
tile is normalized per-partition (reciprocal of its ones column),
transposed back to [dim, token] with a bf16 128x128 PE transpose, and
pushed through a token-major output projection (stationary = y^T tile,
moving = full wo) whose [128 tok, 1024] result streams straight to DRAM
in bf16.

Scheduling: scores/exp are software-pipelined one full chunk ahead of
the PV groups (exp covers BOTH heads per instruction via a two-bank
psum score tile, halving Act per-instruction access overhead).  The
deferred-output pipeline is spread so every cross-engine handoff has a
full PV-group of slack: per-tile normalize (DVE, one tile late) into a
per-chunk ynorm tile; four batched bf16 transposes + one wide yts stage
copy at slot 0 of the next chunk; one output projection per slot during
the next chunk, with its og stage copy on DVE and the out DMA issued
one tile later still so its semaphore wait is pre-satisfied on the SP
queue.  x/qkv projection matmuls are chopped into ~0.4us quanta that a
proportional (error-diffused) pump spreads between attention groups so
the tensor engine stays busy while the scalar engine works through the
exp chain.  The next batch's first-chunk scores are emitted a chunk
before the previous batch ends (they only need its first projection
chunk) so the exp chain never goes cold at batch boundaries.  The last
batch (which has no next-batch
projection quanta to pump) holds an outproj backlog early and drains it
two-per-slot late; its final chunk runs per-tile mini posts that borrow
the then-idle score psum banks and split og staging across Act+DVE, and
the very last tile DMAs its two output halves from separate queues.
"""

import os
import sys
from collections import deque

import numpy as np

for _p in ("/opt/trn_rl_repo",):
    if _p not in sys.path and os.path.isdir(_p):
        sys.path.insert(0, _p)

_B, _T, _C = 4, 2048, 1024
_NHEAD, _HD = 16, 64
_NC = 8
_LOC = (_NHEAD // _NC) * _HD  # feature dims per core = 128 (2 heads)
_BT = _B * _T                 # 8192 tokens
_TC = 512                     # token chunk (psum bank / moving width)
_KC = _C // 128               # 8 contraction chunks over the embedding
_NQC = _T // _TC              # 4 query chunks per batch
_NKB = _T // 128              # 16 key blocks per batch
_EXPM = float(np.exp(-10.0))  # exp of the mask fill value

TRACE = bool(int(os.environ.get("KERNEL_TRACE", "0")))
LAST_EXEC_NS = None
LAST_RESULTS = None

_cache = {}


def _build():
    import concourse.mybir as mybir
    import concourse.tile as tile
    from concourse import bacc

    f32 = mybir.dt.float32
    f32r = mybir.dt.float32r
    bf16 = mybir.dt.bfloat16
    AF = mybir.ActivationFunctionType

    nc = bacc.Bacc("TRN2", target_bir_lowering=False, debug=False)

    xT_d = nc.dram_tensor("xT", [_C, _BT], bf16, kind="ExternalInput").ap()
    wqT_d = nc.dram_tensor("wqT", [_C, _LOC], bf16, kind="ExternalInput").ap()
    wkT_d = nc.dram_tensor("wkT", [_C, _LOC], bf16, kind="ExternalInput").ap()
    wvT_d = nc.dram_tensor("wvT", [_C, _LOC], bf16, kind="ExternalInput").ap()
    woT_d = nc.dram_tensor("woT", [_LOC, _C], bf16, kind="ExternalInput").ap()
    mbig_d = nc.dram_tensor("mbig", [128, 16 * 128], bf16,
                            kind="ExternalInput").ap()
    idr_d = nc.dram_tensor("identr", [128, 128], f32r,
                           kind="ExternalInput").ap()
    outT_d = nc.dram_tensor("outT", [_BT, _C], bf16,
                            kind="ExternalOutput").ap()

    xT_v = xT_d.rearrange("(c p) n -> p c n", p=128)    # [128, 8, 8192]
    wq_v = wqT_d.rearrange("(c p) m -> p c m", p=128)   # [128, 8, 128]
    wk_v = wkT_d.rearrange("(c p) m -> p c m", p=128)
    wv_v = wvT_d.rearrange("(c p) m -> p c m", p=128)
    wo_v = woT_d.rearrange("p (m n) -> p m n", n=128)   # [128, 8, 128]

    with tile.TileContext(nc) as tc:
        with (
            tc.tile_pool(name="consts", bufs=1) as cp,
            tc.tile_pool(name="sb", bufs=2) as sp,
            tc.tile_pool(name="ps", bufs=2, space="PSUM") as pp,
        ):
            # ---------------- constants ----------------
            # (x chunk 0 is DMA'd before the weights -- see main flow --
            # so the first projection matmul can start ~6us earlier)
            w_sb = {}
            for nm in ("q", "k", "v"):
                wt = cp.tile([128, _KC, 128], bf16, tag=f"w{nm}",
                             name=f"w{nm}")
                w_sb[nm] = wt
            wo_sb = cp.tile([128, _KC, 128], bf16, tag="wo")
            mbig = cp.tile([128, 16 * 128], bf16, tag="mbig")
            identr = cp.tile([128, 128], f32r, tag="identr")
            identb = cp.tile([128, 128], bf16, tag="identb")
            # constant 128.0 column: moving operand of the 1-wide
            # masked-denominator matmul
            cnt128 = cp.tile([128, 1], bf16, tag="cnt128")

            def load_consts():
                for nm, v in (("k", wk_v), ("v", wv_v)):
                    nc.sync.dma_start(w_sb[nm][:], v[:])
                nc.sync.dma_start(wo_sb[:], wo_v[:])
                nc.sync.dma_start(mbig[:], mbig_d[:])
                nc.sync.dma_start(identr[:], idr_d[:])
                # bf16 identity for the (faster) bf16 PE transposes
                nc.vector.tensor_copy(identb[:], identr[:].bitcast(f32))
                nc.vector.memset(cnt128[:], 128.0)

            # per-batch activation tiles (written during prev batch's attn)
            def batch_tiles():
                qTb = sp.tile([128, _T], bf16, tag="qTb", bufs=2)
                kTb = sp.tile([128, _T], bf16, tag="kTb", bufs=2)
                # v in [token, dim] layout per 128-token key block:
                # [.., kb, h, 0:64] = v dims, [.., kb, h, 64] = ones
                vab = sp.tile([128, _NKB, 2, 65], bf16, tag="vab", bufs=2)
                nc.vector.memset(vab[:, :, :, 64:65], 1.0)
                return {"q": qTb, "k": kTb, "v": vab}

            # ---------------- interleaved work quanta ----------------
            # Projection chunks (512 tokens) and output-projection drains
            # are emitted as generator steps (~0.4us of PE work each) and
            # pumped from inside the attention loops, keeping the tensor
            # engine busy while the scalar engine runs the exp chain.
            pending = deque()
            sched = {"q": 0, "w": 0}  # pending quanta / remaining weight

            def push_gen(g, n):
                pending.append(g)
                sched["q"] += n

            def pump(n=1):
                for _ in range(n):
                    while pending:
                        try:
                            next(pending[0])
                        except StopIteration:
                            pending.popleft()
                            continue
                        sched["q"] -= 1
                        break
                    else:
                        return

            def pump_w(w):
                # proportional pumping with error diffusion: spread the
                # pending quanta evenly over the remaining weighted slots
                # of this batch (ceil would drain the queue early and
                # leave the last chunk dry)
                W = max(sched["w"], 1)
                sched["c"] = sched.get("c", 0.0) + sched["q"] * w / W
                n = min(sched["q"], int(sched["c"]))
                sched["c"] -= n
                sched["w"] = max(sched["w"] - w, 0)
                pump(n)

            def drain():
                while pending:
                    pump(1)

            def proj_gen(t, bt, xall):
                lt = t % 4
                tok = slice(lt * _TC, (lt + 1) * _TC)
                for nm in ("q", "k"):
                    ps = pp.tile([128, _TC], f32, tag="pbig", bufs=2)
                    for c in range(_KC):
                        nc.tensor.matmul(
                            ps[:],
                            w_sb[nm][:, c, :],
                            xall[:, c, :],
                            start=(c == 0),
                            stop=(c == _KC - 1),
                        )
                        if c % 2 == 1:
                            yield
                    nc.vector.tensor_copy(bt[nm][:, tok], ps[:])
                    yield
                # direct v^T: stationary = x slice, moving = wv chunk ->
                # psum [128 tok, 2, 64] per 128-token block
                ps = pp.tile([128, 4, 2, 64], f32, tag="pbig", bufs=2)
                for tb in range(4):
                    for c in range(_KC):
                        nc.tensor.matmul(
                            ps[:, tb, :, :],
                            xall[:, c, tb * 128:(tb + 1) * 128],
                            w_sb["v"][:, c, :],
                            start=(c == 0),
                            stop=(c == _KC - 1),
                        )
                    yield
                kb0 = lt * 4
                nc.vector.tensor_copy(
                    bt["v"][:, kb0:kb0 + 4, :, 0:64], ps[:])
                yield

            def push_chunk(t, bt, splits=(4, 8)):
                lt = t % 4
                lo = (t // 4) * _T + lt * _TC
                xall = sp.tile([128, _KC, _TC], bf16, tag="xall", bufs=2)
                c0 = 0
                for c1 in splits:
                    nc.sync.dma_start(xall[:, c0:c1, :],
                                      xT_v[:, c0:c1, lo:lo + _TC])
                    c0 = c1
                push_gen(proj_gen(t, bt, xall), 15)

            # the out DMA for each query tile is issued one post later so
            # its semaphore wait is already satisfied at issue time and the
            # SP queue never parks (head-of-line would delay x input DMAs)
            outdma = []

            def flush_outdma():
                while outdma:
                    outdma.pop(0)()

            # Deferred-output pipeline, spread so every cross-engine handoff
            # has a full PV-group of slack and the PE never waits on it:
            #  - make_norm(c,u): recip + per-partition scalar multiply drain
            #    y2 -> ynorm_c (DVE); fired one tile late (slot u+1).
            #  - runa(c): four PE transposes into one psum bank + ONE wide
            #    yts stage copy (DVE); fired at slot 0 of chunk c+1.
            #  - outproj(c,u): ldweights + two 512-wide matmuls + og stage
            #    (Pool) + out DMA (issued one more tile later so its wait is
            #    pre-satisfied on the SP queue); fired one per slot during
            #    chunk c+1 (last one at slot 0 of chunk c+2).
            outdma = []
            outs = deque()
            runa_box = [None]

            def flush_outdma():
                while outdma:
                    outdma.pop(0)()

            def make_norm(c, u, y2, ynorm_c):
                def run():
                    zr = sp.tile([128, 2], f32, tag="zr", bufs=6)
                    for h2 in range(2):
                        nc.vector.reciprocal(
                            zr[:, h2:h2 + 1], y2[h2][:, u, 64:65])
                        nc.vector.tensor_scalar_mul(
                            ynorm_c[:, u, h2 * 64:(h2 + 1) * 64],
                            y2[h2][:, u, 0:64],
                            zr[:, h2:h2 + 1],
                        )
                return run

            def make_outproj(b, c, u, yts, last, uu=None, fin=False):
                su = u if uu is None else uu

                def run():
                    og = sp.tile([128, 2, _TC], bf16, tag="og", bufs=8)
                    for half in range(2):
                        if last:
                            # the score psum banks are idle in the last
                            # chunk: borrow them so the outproj rotation
                            # is not gated by the og drains
                            opsw = pp.tile([128, 2, _TC], f32, tag="sps",
                                           bufs=2, name="opsw")
                            ops = opsw[:, half, :]
                        else:
                            ops = pp.tile([128, _TC], f32, tag="pbig",
                                          bufs=2)
                        nc.tensor.matmul(
                            ops,
                            yts[:, su, :],
                            wo_sb[:, 4 * half:4 * half + 4, :],
                            start=True, stop=True,
                        )
                        if last and half == 0:
                            # Act's queue is empty in the late last batch:
                            # split the two staging copies across Act+DVE
                            nc.scalar.copy(og[:, half, :], ops)
                        else:
                            # gpsimd cannot read PSUM (BIR verifier), so
                            # the staging copies stay on DVE
                            nc.vector.tensor_copy(og[:, half, :], ops)
                    tokr = b * _T + c * _TC + u * 128

                    def dma():
                        nc.sync.dma_start(outT_d[tokr:tokr + 128, :], og[:])
                    if last and fin:
                        # very last tile: DMA each half right away from
                        # its own queue so the final transfers pipeline
                        # instead of serializing on the SP sequencer
                        nc.scalar.dma_start(
                            outT_d[tokr:tokr + 128, 0:_TC], og[:, 0, :])
                        nc.sync.dma_start(
                            outT_d[tokr:tokr + 128, _TC:], og[:, 1, :])
                    elif last:
                        dma()
                    else:
                        outdma.append(dma)
                        if len(outdma) > 1:
                            outdma.pop(0)()
                return run

            def make_runa(b, c, ynorm_c, last):
                def run():
                    yt = pp.tile([128, 4, 128], bf16, tag="pbig", bufs=2)
                    for u in range(4):
                        nc.tensor.transpose(
                            yt[:, u, :], ynorm_c[:, u, :], identb[:])
                    yts = sp.tile([128, 4, 128], bf16, tag="yts", bufs=3)
                    nc.vector.tensor_copy(yts[:], yt[:])
                    for u in range(4):
                        outs.append(make_outproj(b, c, u, yts, last))
                return run

            def mini_post(b, c, u, ynorm_c, fin=False):
                # tail-of-kernel per-tile post: transpose + stage + output
                # projection fired as soon as the tile's normalize is out,
                # so the drain after the last PV group is just one tile
                yt = pp.tile([128, 4, 128], bf16, tag="pbig", bufs=2)
                nc.tensor.transpose(
                    yt[:, 0, :], ynorm_c[:, u, :], identb[:])
                yts = sp.tile([128, 4, 128], bf16, tag="yts", bufs=3)
                nc.vector.tensor_copy(yts[:, 0, :], yt[:, 0, :])
                make_outproj(b, c, u, yts, True, uu=0, fin=fin)()

            # ---------------- attention ----------------
            def emit_score(bt, cc, kb):
                # score block + exp (+ causal mask on diagonal blocks);
                # both heads land in one two-bank psum tile so a single
                # exp instruction covers them (halves Act instruction
                # count and its per-instruction access overhead)
                qTb, kTb = bt["q"], bt["k"]
                j = kb - 4 * cc
                off = 128 * j if j > 0 else 0
                sps = pp.tile([128, 2, _TC], f32, tag="sps", bufs=2)
                for h in range(2):
                    rows = slice(h * 64, (h + 1) * 64)
                    nc.tensor.matmul(
                        sps[:, h, off:],
                        kTb[rows, kb * 128:(kb + 1) * 128],
                        qTb[rows, cc * _TC + off:(cc + 1) * _TC],
                        start=True, stop=True,
                    )
                pexp = sp.tile([128, 2, _TC], bf16, tag="pexp", bufs=32)
                nc.scalar.activation(
                    pexp[:, :, off:], sps[:, :, off:], AF.Exp, scale=0.125,
                )
                if j >= 0:
                    # diagonal 128-block: keep where q >= k, else
                    # fill exp(-10)
                    for h in range(2):
                        nc.gpsimd.affine_select(
                            out=pexp[:, h, 128 * j:128 * (j + 1)],
                            in_=pexp[:, h, 128 * j:128 * (j + 1)],
                            compare_op=mybir.AluOpType.is_ge,
                            fill=_EXPM,
                            base=0,
                            pattern=[[1, 128]],
                            channel_multiplier=-1,
                        )
                return pexp

            def batch_prologue(bt):
                # the first chunk's scores, emitted in the PREVIOUS
                # batch's tail so the exp chain never goes cold across
                # batch boundaries
                pex = {kb: emit_score(bt, 0, kb) for kb in range(4)}
                return pex

            def attn_batch(b, bt, bt_next, pro):
                vab = bt["v"]
                pex = pro
                sched["w"] = _NQC * (8 * 2 + 4)
                # chunk-pipelined: scores/exp for chunk c+1 are emitted
                # while chunk c's PV groups accumulate, so the exp chain
                # is a full chunk ahead of its consumers.
                last_b = b == _B - 1
                pro_next = None
                for c in range(_NQC):
                    nkb = 4 * c + 4
                    nxt = list(range(nkb + 4)) if c < 3 else []
                    pex_next = {}
                    emitted = 0
                    norm_q = []
                    y2 = []
                    for h in range(2):
                        t = pp.tile([128, 4, 128], f32, tag="y2", bufs=2,
                                    name=f"y2_{b}_{c}_{h}")
                        y2.append(t)
                    ynorm_c = sp.tile([128, 4, 128], bf16, tag="yn", bufs=3,
                                      name=f"yn_{b}_{c}")
                    for u in range(4):
                        qt = 4 * c + u
                        for h in range(2):
                            for kb in range(4 * c + u + 1):
                                nc.tensor.matmul(
                                    y2[h][:, u, 0:65],
                                    pex[kb][:, h, u * 128:(u + 1) * 128],
                                    vab[:, kb, h, :],
                                    start=(kb == 0),
                                    stop=False,
                                )
                            # the fully-masked key blocks' softmax mass is
                            # exp(-10)*128*#blocks -- a constant added to
                            # the ones (denominator) column via a 1-wide
                            # matmul (mbig rows kb>qt hold exp(-10); the
                            # moving operand is a constant 128).  Their
                            # V-weighted numerator term (~2e-3 absolute)
                            # is dropped.
                            nc.tensor.matmul(
                                y2[h][:, u, 64:65],
                                mbig[:, qt * 128:(qt + 1) * 128],
                                cnt128[:, :],
                                start=False, stop=True,
                            )
                            share = (len(nxt) * (2 * u + h + 1)) // 8
                            while emitted < share:
                                pex_next[nxt[emitted]] = emit_score(
                                    bt, c + 1, nxt[emitted])
                                emitted += 1
                            pump_w(2)
                        # slot actions: previous chunk's outproj (one per
                        # slot), its transposes at slot 0, and this
                        # chunk's normalize one tile late.  In the last
                        # batch, hold a backlog early on so the late
                        # chunks (which have no next-batch projections to
                        # pump) still have PE filler.
                        if last_b and c < 2:
                            if len(outs) > 2:
                                outs.popleft()()
                        elif last_b and c >= 2:
                            outs.popleft()()
                            if outs and (u % 2 == 1):
                                outs.popleft()()
                        elif outs:
                            outs.popleft()()
                        if u == 0 and runa_box[0] is not None:
                            runa_box[0]()
                            runa_box[0] = None
                        tailmode = last_b and c == _NQC - 1
                        if norm_q:
                            norm_q.pop(0)()
                            if tailmode and u >= 1:
                                mini_post(b, c, u - 1, ynorm_c)
                        norm_q.append(make_norm(c, u, y2, ynorm_c))
                        pump_w(1)
                    # chunk end: the last tile's normalize must be emitted
                    # before the next chunk re-acquires the y2 buffers
                    while norm_q:
                        norm_q.pop(0)()
                    if not (last_b and c == _NQC - 1):
                        runa_box[0] = make_runa(b, c, ynorm_c,
                                                last_b and c == _NQC - 2)
                    pex = pex_next
                    if bt_next is not None and c < 3:
                        push_chunk(4 * (b + 1) + c + 1, bt_next)
                    if bt_next is not None and c == 2:
                        # the next batch's first-chunk scores only need
                        # its first projection chunk (pushed at this
                        # batch's start and long since pumped), so the
                        # prologue can emit a chunk early instead of
                        # serializing behind the drain
                        pro_next = batch_prologue(bt_next)
                if bt_next is not None:
                    drain()
                else:
                    # tail of the last batch: flush the remaining carried
                    # output projections and the final tile's mini post
                    while outs:
                        outs.popleft()()
                    mini_post(b, _NQC - 1, 3, ynorm_c, fin=True)
                return pro_next

            # ---------------- main flow ----------------
            bt = batch_tiles()
            nc.sync.dma_start(w_sb["q"][:, 0:4, :], wq_v[:, 0:4, :])
            xall0 = sp.tile([128, _KC, _TC], bf16, tag="xall", bufs=2,
                            name="xall0")
            nc.sync.dma_start(xall0[:, 0:1, :], xT_v[:, 0:1, 0:_TC])
            nc.sync.dma_start(xall0[:, 1:4, :], xT_v[:, 1:4, 0:_TC])
            nc.sync.dma_start(w_sb["q"][:, 4:8, :], wq_v[:, 4:8, :])
            nc.sync.dma_start(xall0[:, 4:8, :], xT_v[:, 4:8, 0:_TC])
            push_gen(proj_gen(0, bt, xall0), 15)
            load_consts()
            for t in range(1, 4):
                push_chunk(t, bt)
            drain()
            pro = batch_prologue(bt)
            for b in range(_B):
                bt_next = batch_tiles() if b < _B - 1 else None
                if bt_next is not None:
                    push_chunk(4 * (b + 1), bt_next)
                pro = attn_batch(b, bt, bt_next, pro)
                bt = bt_next
            drain()
            flush_outdma()

    nc.compile()
    return nc, outT_d.name


def _get_nc():
    if "nc" not in _cache:
        _cache["nc"] = _build()
    return _cache["nc"]


def _consts():
    import ml_dtypes

    mbig = np.zeros((128, 16 * 128), dtype=np.float32)
    for kb in range(16):
        for qt in range(16):
            if kb > qt:
                mbig[kb, qt * 128:(qt + 1) * 128] = _EXPM
    identr = np.eye(128, dtype=np.float32)
    return (
        mbig.astype(ml_dtypes.bfloat16),
        identr,
    )


def kernel(**inputs):
    import ml_dtypes

    from concourse.bass_utils import run_bass_kernel_spmd

    x = np.ascontiguousarray(np.asarray(inputs["x"]), dtype=np.float32)
    wq = np.ascontiguousarray(np.asarray(inputs["wq"]), dtype=np.float32)
    wk = np.ascontiguousarray(np.asarray(inputs["wk"]), dtype=np.float32)
    wv = np.ascontiguousarray(np.asarray(inputs["wv"]), dtype=np.float32)
    wo = np.ascontiguousarray(np.asarray(inputs["wo"]), dtype=np.float32)

    bf16 = ml_dtypes.bfloat16
    xT = np.ascontiguousarray(x.reshape(_BT, _C).T).astype(bf16)
    mbig, identr = _consts()

    in_maps = []
    for i in range(_NC):
        r = slice(_LOC * i, _LOC * (i + 1))
        in_maps.append({
            "xT": xT,
            "wqT": np.ascontiguousarray(wq[r].T).astype(bf16),
            "wkT": np.ascontiguousarray(wk[r].T).astype(bf16),
            "wvT": np.ascontiguousarray(wv[r].T).astype(bf16),
            "woT": np.ascontiguousarray(wo[:, r].T).astype(bf16),
            "mbig": mbig,
            "identr": identr,
        })

    nc, outname = _get_nc()
    try:
        res = run_bass_kernel_spmd(nc, in_maps, list(range(_NC)), trace=TRACE)
    except ModuleNotFoundError:
        # NTFF profiling hook unavailable in this container
        res = run_bass_kernel_spmd(nc, in_maps, list(range(_NC)), trace=False)

    global LAST_EXEC_NS, LAST_RESULTS
    LAST_EXEC_NS = res.exec_time_ns
    LAST_RESULTS = res

    acc = np.zeros((_BT, _C), dtype=np.float64)
    for i in range(_NC):
        acc += np.asarray(res.results[i][outname], dtype=np.float64)
    return acc.reshape(_B, _T, _C).astype(np.float32)

